# revision 95
# baseline (speedup 1.0000x reference)
"""AttentionPooling Trainium2 Bass kernel.

Problem (hardcoded shapes): B=64, T=4096, D=256, U=64
    uit    = tanh(inputs @ w + b)          # [B,T,U]
    scores = (uit @ u)[..., 0]             # [B,T]
    scores += (1-mask) * -1e9
    attn   = softmax(scores, axis=1)       # [B,T]
    out    = einsum('btd,bt->bd', inputs, attn)

Sharding: pure data-parallel, 8 examples per core across 8 NeuronCores.

Per-core design (EX=8 examples). The DMA pipe is the roofline (32 MiB of
f32 x reads ~= 93 us at 360 GB/s), so nothing else may ride it: the old
xbar DMA transpose is replaced by PE transposes (bf16 in/out of PSUM)
with a DVE copy back to SBUF.

  - x loaded HBM->SBUF with f32->bf16 cast (SWDGE), natural layout, in
    (h, jh) quarters: x2[p, h, jh, j16, d'] = x[128*(16jh+j16)+p, 128h+d'];
    the very first chunk rides HWDGE as f32 + DVE cast to start earlier
  - PE transpose per 128x128 tile via identity (bf16 PSUM out),
    grouped 8 tiles = 1 PSUM bank per (Q, h); DVE copies bank -> SBUF
  - comp1 (PE):  z^T tiles = w_h^T @ x^T_h accumulated over h,
                 2 T-groups packed per PSUM bank via partition halves
  - tanh (ACT):  uitT = tanh(z + b), per-partition bias, bf16 out
  - comp2 (PE):  scores^T via uitT chunks stationary vs block-diag u,
                 both halves packed in one PSUM bank [128, 32]; each
                 col-pair group ends with a penalty matmul folding
                 1e9*(m-1) into the scores ((m-1) stationary vs a
                 1e9*I128 column slice), so no mask transpose is needed
  - softmax:     exp on ACT emits masked e directly, with per-quarter
                 row sums via accum_out; quarter sums reduced + cast on
                 DVE; denom broadcast to all partitions via ones-matmul
                 on PE; reciprocal on DVE
  - comp4 (PE):  unnormalized context accumulated with x2 tiles as
                 STATIONARY and e columns as moving (N=1, so ~free in
                 moving-column terms) -> psum [128 d', 2 h]
  - final (DVE): context * (1/denom) per-partition scale chained right
                 behind the reciprocal on the DVE queue, f32 -> DMA out

Emission is software-pipelined over tickets t=(ex, Q), Q in 0..3, with
stages skewed across iterations so each engine queue never head-blocks:
  iter t: comp2/softmax/comp4(t-2) | scale+store(t-3) | comp1+tanh(t-1)
          | transpose(t)
All x loads are emitted up front (the DMA pipe is the critical resource
and runs gapless); the last two examples load at T-quarter granularity,
and the final example's last 8-chunk group is processed as two 4-chunk
sub-tickets (half-width transpose/copy/comp1/tanh on the closing chain,
with a shared 8-wide exp since exp cost is init-dominated). The closing
sub-tickets' transposes are hand-scheduled ahead of the preceding
comp1 on the PE queue, with an extra PSUM transpose buffer (tp=3,
pcr=1) so back-to-back sub-tickets don't hit a slot-recycling WAR.
Context stores for ex0..6 are batched and gated on the final x load so
they ride the tail's idle DMA window instead of delaying the x stream.
"""

import numpy as np

B, T, D, U = 64, 4096, 256, 64
NCORES = 8
EX = B // NCORES  # 8 examples per core
NJ = T // 128     # 32 T-chunks
NEG_BIG = -1e9

_CACHE = {}


def _build(xt_bufs=6, up_bufs=3, sp_bufs=2, cx_bufs=2):
    """Build and compile the per-core Bass program."""
    import concourse.bacc as bacc
    import concourse.tile as tile
    import concourse.mybir as mybir
    from concourse._compat import axon_active
    from concourse.tile import add_dep_helper

    f32 = mybir.dt.float32
    bf16 = mybir.dt.bfloat16
    i32 = mybir.dt.int32
    AF = mybir.ActivationFunctionType
    ALU = mybir.AluOpType

    nc = bacc.Bacc("TRN2", target_bir_lowering=False, debug=not axon_active())

    x_d = nc.dram_tensor("x", (EX, T, D), f32, kind="ExternalInput").ap()
    mask_d = nc.dram_tensor("mask", (EX, T), i32, kind="ExternalInput").ap()
    w_d = nc.dram_tensor("w", (D, U), f32, kind="ExternalInput").ap()
    b_d = nc.dram_tensor("b", (U,), f32, kind="ExternalInput").ap()
    u_d = nc.dram_tensor("u", (U, 1), f32, kind="ExternalInput").ap()
    out_d = nc.dram_tensor("out", (EX, D), f32, kind="ExternalOutput").ap()

    NT = EX * 4  # tickets: (ex, Q)

    with tile.TileContext(nc) as tc:
        with (
            tc.tile_pool(name="cp", bufs=1) as cp,
            tc.tile_pool(name="xp", bufs=EX) as xp,
            tc.tile_pool(name="xt", bufs=xt_bufs) as xt,
            tc.tile_pool(name="up", bufs=up_bufs) as up,
            tc.tile_pool(name="sp", bufs=sp_bufs) as sp,
            tc.tile_pool(name="cx", bufs=cx_bufs) as cx,
            tc.tile_pool(name="tp", bufs=3, space="PSUM") as tp,
            tc.tile_pool(name="pzp", bufs=2, space="PSUM") as pzp,
            tc.tile_pool(name="psp", bufs=2, space="PSUM") as psp,
            tc.tile_pool(name="pcp", bufs=1, space="PSUM") as pcp,
        ):
            # ---- first x chunk rides HWDGE ahead of the consts ----
            x0s = cp.tile([128, 8, 128], f32, tag="x0s")
            nc.sync.dma_start(
                out=x0s,
                in_=x_d[0, 0:1024, 0:128].rearrange("(j p) d -> p j d", p=128),
            )

            # ---- constants (sync/HWDGE loads + DVE fixups; the Pool
            # queue is reserved for the x-load SWDGE generation) ----
            w_raw = cp.tile([128, 2, U], f32, tag="wr")
            nc.sync.dma_start(
                out=w_raw, in_=w_d.rearrange("(c p) u -> p c u", p=128)
            )
            w_bf = cp.tile([128, 2, U], bf16, tag="w")
            nc.vector.tensor_copy(out=w_bf, in_=w_raw)

            b_sb = cp.tile([128, 1], f32, tag="b")
            b_2d = b_d.rearrange("(u o) -> u o", o=1)
            nc.sync.dma_start(out=b_sb[0:U, :], in_=b_2d)
            nc.sync.dma_start(out=b_sb[U:128, :], in_=b_2d)

            u_f = cp.tile([128, 1], f32, tag="uf")
            nc.sync.dma_start(out=u_f[0:U, :], in_=u_d)
            nc.sync.dma_start(out=u_f[U:128, :], in_=u_d)
            u_bd = cp.tile([128, 2], bf16, tag="u")
            nc.vector.memset(u_bd, 0.0)
            nc.vector.tensor_copy(out=u_bd[0:U, 0:1], in_=u_f[0:U, :])
            nc.vector.tensor_copy(out=u_bd[U:128, 1:2], in_=u_f[U:128, :])

            ones_sq = cp.tile([128, 128], bf16, tag="ones")
            nc.vector.memset(ones_sq, 1.0)

            # identity for PE transposes: out[x, y] = (x - y) != 0 ? 0 : 1
            ident = cp.tile([128, 128], bf16, tag="ident")
            nc.vector.memset(ident, 0.0)
            nc.gpsimd.affine_select(
                out=ident,
                in_=ident,
                compare_op=ALU.not_equal,
                fill=1.0,
                base=0,
                pattern=[[-1, 128]],
                channel_multiplier=1,
            )

            # mask: contiguous [2][128q, 128p] i32 with
            # mi[L][q, p] = mask[4L + q//32, 128*(q%32) + p].
            # Stored as (m - 1) in bf16: a matmul against a column slice
            # of bigi = 1e9*I128 folds the -1e9*(1-m) penalty into scores.
            mask3 = mask_d.rearrange("e t -> (e t)").rearrange(
                "(L q p) -> L q p", L=2, q=128
            )
            mbm1 = []
            for L in range(2):
                mi_ = cp.tile([128, 128], i32, tag=f"mi{L}")
                nc.sync.dma_start(out=mi_, in_=mask3[L])
                mb_ = cp.tile([128, 128], bf16, tag=f"mb{L}")
                nc.vector.tensor_scalar_add(mb_, mi_, -1.0)
                mbm1.append(mb_)

            bigi = cp.tile([128, 128], bf16, tag="bigi")
            nc.vector.memset(bigi, 0.0)
            nc.gpsimd.affine_select(
                out=bigi,
                in_=bigi,
                compare_op=ALU.not_equal,
                fill=1e9,
                base=0,
                pattern=[[-1, 128]],
                channel_multiplier=1,
            )

            # ---- all x loads up front: the DMA pipe is the bottleneck
            # and must never wait on anything. The very first chunk rides
            # HWDGE as f32 (starts ~1.3us vs ~2.4us for SWDGE gen) and is
            # cast to bf16 on the then-idle DVE; casts otherwise require
            # SWDGE. The last two examples load at T-quarter granularity
            # (the final quarter split again) so the endgame tickets wait
            # on the smallest possible piece of the stream. ----
            x2s = []
            for ex in range(EX):
                x2 = xp.tile([128, 2, 2, 16, 128], bf16, tag="x2")
                x2s.append(x2)

            load_insts = []

            def load_piece(ex, h, j0, nj):
                jh, jj = divmod(j0, 16)
                i_ = nc.gpsimd.dma_start(
                    out=x2s[ex][:, h, jh, jj : jj + nj],
                    in_=x_d[
                        ex, 128 * j0 : 128 * (j0 + nj),
                        128 * h : 128 * (h + 1),
                    ].rearrange("(j p) d -> p j d", p=128),
                )
                load_insts.append(i_)

            nc.vector.tensor_copy(out=x2s[0][:, 0, 0, 0:8], in_=x0s)
            load_piece(0, 1, 0, 8)
            load_piece(0, 0, 8, 8)
            load_piece(0, 1, 8, 8)
            load_piece(0, 0, 16, 16)
            load_piece(0, 1, 16, 16)
            for ex in range(1, EX):
                if ex < EX - 2:
                    pieces = [(0, 16), (16, 16)]
                elif ex < EX - 1:
                    pieces = [(0, 8), (8, 8), (16, 8), (24, 8)]
                else:
                    pieces = [(0, 8), (8, 8), (16, 8), (24, 4), (28, 4)]
                for j0, nj in pieces:
                    for h in range(2):
                        load_piece(ex, h, j0, nj)

            # ---- software-pipelined compute ----
            xt_map = {}
            uit_map = {}
            ps_map = {}
            e_map = {}
            e1_map = {}
            pcr_map = {}

            # variable-width tickets: (ex, jb, nj, si, nsl, last). ex0..6
            # run 4x 8-chunk tickets; ex7 splits its final group into two
            # 4-chunk sub-tickets so every stage of the closing dependency
            # chain (transpose/copy/comp1/tanh/exp) is half-width.
            def mk(ex, jb, nj, last):
                # exp groups stay 8-wide (their cost is init-dominated):
                # a leading 4-wide sub-ticket defers its exp to the
                # trailing one, which covers the whole 8-j group
                if nj == 4 and jb % 8 == 0:
                    eg = None
                else:
                    eg = (8 * (jb // 8), 8, jb // 8)
                return dict(ex=ex, jb=jb, nj=nj, eg=eg, last=last)

            tickets = []
            for ex in range(EX):
                widths = [8, 8, 8, 8] if ex < EX - 1 else [8, 8, 8, 4, 4]
                jb = 0
                for si, nj in enumerate(widths):
                    tickets.append(mk(ex, jb, nj, si == len(widths) - 1))
                    jb += nj
            NTK = len(tickets)
            ex_tickets = {}
            for tk in tickets:
                ex_tickets.setdefault(tk["ex"], []).append(tk)

            def stage_a(i):
                # PE transposes of this ticket's x-tiles; DVE copy-out
                tk = tickets[i]
                ex, jb, nj = tk["ex"], tk["jb"], tk["nj"]
                x2 = x2s[ex]
                tps = []
                for h in range(2):
                    tp_ = tp.tile([128, 8, 128], bf16, tag="tp")
                    for jj in range(nj):
                        j = jb + jj
                        nc.tensor.transpose(
                            tp_[:, jj, :], x2[:, h, j // 16, j % 16, :], ident
                        )
                    tps.append(tp_)
                # wide tickets: copies split per pi-block so comp1's pi0
                # matmuls start early; the final example's tickets use one
                # fused copy per h (init-dominated, shortens the tail)
                hb = nj // 2
                xts = []
                for h in range(2):
                    xt_ = xt.tile([128, 8, 128], bf16, tag="xt")
                    # fused copies where the ticket's tanh gates the
                    # endgame PE queue (measured per-ticket): narrow
                    # closing sub-tickets, ex7's third group, ex6's last
                    if nj < 8 or (ex == EX - 1 and jb == 16) or (ex == EX - 2 and jb == 24):
                        nc.vector.tensor_copy(
                            out=xt_[:, 0:nj, :], in_=tps[h][:, 0:nj, :]
                        )
                    else:
                        for pi in range(2):
                            nc.vector.tensor_copy(
                                out=xt_[:, hb * pi : hb * (pi + 1), :],
                                in_=tps[h][:, hb * pi : hb * (pi + 1), :],
                            )
                    xts.append(xt_)
                xt_map[i] = xts

            def stage_b(i):
                # comp1 (z^T for 2 T-groups packed in partition halves) + tanh
                tk = tickets[i]
                nj = tk["nj"]
                hb = nj // 2
                w = 64 * nj
                pz = pzp.tile([128, 512], f32, tag="pz")
                xts = xt_map.pop(i)
                for pi in range(2):
                    for h in range(2):
                        nc.tensor.matmul(
                            out=pz[64 * pi : 64 * pi + 64, 0:w],
                            lhsT=w_bf[:, h, :],
                            rhs=xts[h][:, hb * pi : hb * (pi + 1), :],
                            start=(h == 0),
                            stop=(h == 1),
                        )
                uitQ = up.tile([128, 512], bf16, tag="uit")
                nc.scalar.activation(
                    out=uitQ[:, 0:w], in_=pz[:, 0:w], func=AF.Tanh,
                    bias=b_sb, scale=1.0,
                )
                uit_map[i] = uitQ

            def stage_c(i):
                # comp2 -> scores^T; per-ticket masked exp; on the last
                # ticket of an example the denominator + unnormalized
                # context accumulation (comp4)
                tk = tickets[i]
                ex, jb, nj = tk["ex"], tk["jb"], tk["nj"]
                hb = nj // 2
                uitQ = uit_map.pop(i)
                if jb == 0:
                    ps_map[ex] = psp.tile([128, 32], f32, tag="ps", name="ps")
                    e1_map[ex] = sp.tile([128, 8], f32, tag="e1q", name="e1q")
                ps = ps_map[ex]
                # each col-pair group: scores matmul, then a penalty matmul
                # folding ps[t', c] += 1e9 * (m - 1) into the same group
                q0 = 32 * (ex % 4)
                for cq in range(hb):
                    c0 = jb + cq
                    nc.tensor.matmul(
                        out=ps[:, c0 : c0 + hb + 1 : hb],
                        lhsT=uitQ[:, 128 * cq : 128 * cq + 128],
                        rhs=u_bd,
                        start=True,
                        stop=False,
                    )
                    nc.tensor.matmul(
                        out=ps[:, c0 : c0 + hb + 1 : hb],
                        lhsT=mbm1[ex // 4],
                        rhs=bigi[:, q0 + c0 : q0 + c0 + hb + 1 : hb],
                        start=False,
                        stop=True,
                    )
                # exp of masked scores gives e directly; its accum_out
                # gives this group's row sums
                if tk["eg"] is not None:
                    gb, gn, gsi = tk["eg"]
                    e_ = sp.tile([128, 8], bf16, tag=f"e{gsi}", name=f"e{gsi}")
                    nc.scalar.activation(
                        out=e_[:, 0:gn],
                        in_=ps[:, gb : gb + gn],
                        func=AF.Exp,
                        accum_out=e1_map[ex][:, gsi : gsi + 1],
                    )
                    e_map[(ex, gb)] = e_
                if tk["last"]:
                    ps_map.pop(ex)
                    e1q = e1_map.pop(ex)
                    e1b = sp.tile([128, 1], bf16, tag="e1b")
                    # bf16 round here matches the pre-matmul cast it replaces
                    with nc.allow_low_precision("quarter-sum reduce"):
                        nc.vector.tensor_reduce(
                            out=e1b, in_=e1q[:, 0:4],
                            axis=mybir.AxisListType.X, op=ALU.add,
                        )
                    pcr = pcp.tile([128, 4], f32, tag="pcr")
                    # comp4 first on the PE queue: it only waits on the e
                    # tiles, while pr waits on the slower reduce+cast path
                    x2 = x2s[ex]
                    for h in range(2):
                        for j in range(NJ):
                            nc.tensor.matmul(
                                out=pcr[:, h : h + 1],
                                lhsT=x2[:, h, j // 16, j % 16, :],
                                rhs=e_map[(ex, 8 * (j // 8))][:, j % 8 : j % 8 + 1],
                                start=(j == 0),
                                stop=(j == NJ - 1),
                            )
                    for gb in range(0, NJ, 8):
                        e_map.pop((ex, gb))
                    # denom replicated to all 128 partitions in one matmul
                    nc.tensor.matmul(
                        out=pcr[:, 2:3], lhsT=ones_sq, rhs=e1b,
                        start=True, stop=True,
                    )
                    rr = sp.tile([128, 1], f32, tag="rr")
                    nc.vector.reciprocal(out=rr, in_=pcr[:, 2:3])
                    pcr_map[ex] = (pcr, rr)

            ctx_all = cp.tile([128, EX, 2], f32, tag="ctx_all")

            def stage_d(ex):
                # scale on DVE: it chains directly behind the reciprocal on
                # the same queue, skipping a cross-engine hop before the
                # final store
                pcr, rr = pcr_map.pop(ex)
                nc.vector.tensor_scalar_mul(ctx_all[:, ex, :], pcr[:, 0:2], rr)

            # steady-state skewed pipeline up to the last three tickets
            for it in range(NTK - 2):
                if 0 <= it - 2 < NTK:
                    stage_c(it - 2)
                if 0 <= it - 3 < NTK and tickets[it - 3]["last"]:
                    stage_d(tickets[it - 3]["ex"])
                if 0 <= it - 1 < NTK:
                    stage_b(it - 1)
                stage_a(it)

            # hand-scheduled endgame: both closing sub-tickets' transposes
            # are emitted before either comp1 so the PE queue never makes
            # sub-ticket A's compute wait behind sub-ticket B's data
            stage_c(NTK - 4)
            stage_b(NTK - 3)
            stage_a(NTK - 2)
            stage_a(NTK - 1)
            stage_c(NTK - 3)
            stage_b(NTK - 2)
            stage_b(NTK - 1)
            stage_c(NTK - 2)
            stage_c(NTK - 1)
            stage_d(tickets[NTK - 1]["ex"])

            # stores for ex0..6 are gated on the last x load so they ride
            # the tail's idle DMA window instead of delaying the x stream;
            # ex7's store stays on the critical path
            for h in range(2):
                st_a = nc.sync.dma_start(
                    out=out_d[0 : EX - 1, 128 * h : 128 * (h + 1)].rearrange(
                        "e d -> d e"
                    ),
                    in_=ctx_all[:, 0 : EX - 1, h],
                )
                add_dep_helper(
                    st_a.ins, load_insts[-1].ins,
                    reason="defer early stores past the x stream",
                )
            nc.sync.dma_start(
                out=out_d[EX - 1].rearrange("(h d) -> d h", d=128),
                in_=ctx_all[:, EX - 1, :],
            )

    nc.compile()
    return nc


def _get_nc(**kw):
    key = tuple(sorted(kw.items()))
    if key not in _CACHE:
        _CACHE[key] = _build(**kw)
    return _CACHE[key]


BEST_CFG = dict(xt_bufs=6, up_bufs=3, sp_bufs=2, cx_bufs=2)


def kernel(inputs, mask, w, b, u):
    from concourse.bass_utils import run_bass_kernel_spmd

    nc = _get_nc(**BEST_CFG)
    x = np.ascontiguousarray(np.asarray(inputs, dtype=np.float32))
    m = np.ascontiguousarray(np.asarray(mask, dtype=np.int32))
    wf = np.ascontiguousarray(np.asarray(w, dtype=np.float32))
    bf = np.ascontiguousarray(np.asarray(b, dtype=np.float32))
    uf = np.ascontiguousarray(np.asarray(u, dtype=np.float32))

    in_maps = []
    for c in range(NCORES):
        sl = slice(c * EX, (c + 1) * EX)
        in_maps.append(
            {"x": x[sl], "mask": m[sl], "w": wf, "b": bf, "u": uf}
        )
    res = run_bass_kernel_spmd(nc, in_maps, core_ids=list(range(NCORES)))
    out = np.concatenate([res.results[c]["out"] for c in range(NCORES)], axis=0)
    return out.astype(np.float32)
